# revision 5
# baseline (speedup 1.0000x reference)
"""H2GCNConv on 8 trn2 NeuronCores (Bass/Tile).

Nodes dst-sharded 6250/core; edges partitioned by destination. One SPMD
program computes a mean-aggregation hop (dma_gather chunks <=1920 idxs with
lo/hi int16 source split, dma_scatter_add into a 4-slot-expanded accumulator
so indices are unique per scatter instruction — HBM scatter-add RMW races on
duplicates, verified on HW), folds slots + multiplies 1/deg on DVE, and runs
the final linear on PE. The program runs twice: run 1 produces hop1 shards
(its linear output is discarded), the host concatenates shards (pure data
movement), run 2 consumes hop1 as gather source and emits the final output.
"""
import sys
sys.path.insert(0, "/opt/trn_rl_repo")
import numpy as np
import concourse.bass as bass
import concourse.bacc as bacc
import concourse.tile as tile
mybir = bass.mybir
from concourse.bass_utils import run_bass_kernel_spmd

N, D, E, P = 50000, 128, 600000, 8
SH = N // P
S = 32512                        # lo/hi split for int16 gather indices
NSLOT = 4
ARows = 6304
ACC_ROWS = NSLOT * ARows         # 25216 < 32767
TRASH = 6272
CHUNK_MAX = 1024   # largest dma_gather size verified crash-free on this setup
XA_LO = S + 1                    # aug layout: [rows 0..S-1; zeros; rows S..N-1; zeros]
XA_ROWS = N + 2
NT = 49

_CACHE = {}


def _wrap_idx(a):
    a = np.asarray(a, dtype=np.int16)
    n = a.shape[0]
    w = a.reshape(n // 16, 16).T.copy()
    return np.tile(w, (8, 1))


def _aug(full):
    """[N, D] -> augmented gather source with zero pad rows."""
    out = np.zeros((XA_ROWS, D), np.float32)
    out[0:S] = full[0:S]
    out[XA_LO:XA_LO + (N - S)] = full[S:N]
    return out


def _prep(edge_index):
    src = np.asarray(edge_index[0], dtype=np.int64)
    dst = np.asarray(edge_index[1], dtype=np.int64)
    deg = np.bincount(dst, minlength=N).astype(np.float32)
    inv_deg = (1.0 / np.maximum(deg, 1.0)).astype(np.float32)

    core_of = dst // SH
    order = np.argsort(dst, kind="stable")
    dsorted = dst[order]
    starts = np.searchsorted(dsorted, np.arange(N))
    rank_sorted = np.arange(E) - starts[dsorted]
    rank = np.empty(E, np.int64); rank[order] = rank_sorted
    sr = rank // NSLOT
    slot = rank % NSLOT
    half = (src >= S).astype(np.int64)
    n_sr = int(sr.max()) + 1

    key = core_of * (2 * n_sr) + sr * 2 + half
    ordk = np.argsort(key, kind="stable")
    ks = key[ordk]
    bounds = np.searchsorted(ks, np.arange(P * n_sr * 2 + 1))
    lists = [[[None, None] for _ in range(n_sr)] for _ in range(P)]
    for c in range(P):
        for t in range(n_sr):
            for h in (0, 1):
                k = c * (2 * n_sr) + t * 2 + h
                lists[c][t][h] = ordk[bounds[k]:bounds[k + 1]]

    sizes = [[max(len(lists[c][t][h]) for c in range(P)) for h in (0, 1)]
             for t in range(n_sr)]
    gidx = [[] for _ in range(P)]
    sidx = [[] for _ in range(P)]
    chunks = []
    for t in range(n_sr):
        for h in (0, 1):
            n_pad = -(-max(sizes[t][h], 1) // CHUNK_MAX) * CHUNK_MAX
            for c in range(P):
                el = lists[c][t][h]
                gs = src[el] - (S if h else 0)
                ss = (dst[el] - c * SH) + slot[el] * ARows
                npad = n_pad - len(el)
                gpad = np.full(npad, S if h == 0 else (N - S), np.int64)
                spad = TRASH + (np.arange(npad) % 24)
                gidx[c].append(np.concatenate([gs, gpad]))
                sidx[c].append(np.concatenate([ss, spad]))
            off = 0
            while off < n_pad:
                n = min(CHUNK_MAX, n_pad - off)
                chunks.append((h, n))
                off += n
    gidx = [np.concatenate(g) for g in gidx]
    sidx = [np.concatenate(s) for s in sidx]

    invc = []
    for c in range(P):
        v = np.zeros(NT * 128, np.float32)
        v[:SH] = inv_deg[c * SH:(c + 1) * SH]
        invc.append(v.reshape(NT, 128).T.copy())
    return dict(chunks=chunks, gidx=gidx, sidx=sidx, invc=invc,
                inv_deg=inv_deg)


def _build(chunks, total_idx):
    nc = bacc.Bacc(None, target_bir_lowering=False, debug=False)
    dt = mybir.dt.float32
    i16 = mybir.dt.int16
    CID = total_idx // 16

    srca = nc.dram_tensor("srca", [XA_ROWS, D], dt, kind="ExternalInput")
    x_sl = nc.dram_tensor("x_sl", [6272, D], dt, kind="ExternalInput")
    h1_sl = nc.dram_tensor("h1_sl", [6272, D], dt, kind="ExternalInput")
    g_h = nc.dram_tensor("g_h", [128, CID], i16, kind="ExternalInput")
    s_h = nc.dram_tensor("s_h", [128, CID], i16, kind="ExternalInput")
    inv_h = nc.dram_tensor("inv_h", [128, NT], dt, kind="ExternalInput")
    wt_h = nc.dram_tensor("wt_h", [3 * D, D], dt, kind="ExternalInput")
    bias_h = nc.dram_tensor("bias_h", [128, D], dt, kind="ExternalInput")
    ident_h = nc.dram_tensor("ident_h", [128, 128], dt, kind="ExternalInput")
    hop_h = nc.dram_tensor("hop_sl", [6272, D], dt, kind="ExternalOutput")
    out_h = nc.dram_tensor("out_sl", [6272, D], dt, kind="ExternalOutput")
    acc = nc.dram_tensor("acc", [ACC_ROWS, D], dt)

    def gate(*deps):
        n = None
        for d in deps:
            if d is None:
                continue
            n = nc.gpsimd.nop()
            bass._add_dep_helper(n.ins, d.ins, sync=True, reason="gate")
        return n

    with tile.TileContext(nc) as tc:
        with tc.tile_pool(name="pc", bufs=1) as pc, \
             tc.tile_pool(name="gp", bufs=3) as gp, \
             tc.tile_pool(name="hp", bufs=3) as hp, \
             tc.tile_pool(name="pp", bufs=4, space="PSUM") as pp:
            gix = pc.tile([128, CID], i16)
            six = pc.tile([128, CID], i16)
            dg1 = nc.sync.dma_start(out=gix[:], in_=g_h[:])
            dg2 = nc.sync.dma_start(out=six[:], in_=s_h[:])
            inv_t = pc.tile([128, NT], dt)
            nc.sync.dma_start(out=inv_t[:], in_=inv_h[:])
            zt = pc.tile([128, 2048], dt)
            nc.vector.memset(zt[:], 0.0)

            zds = []
            flat = acc[:].rearrange("r d -> (r d)").rearrange("(p f) -> p f", p=128)
            total = ACC_ROWS * D // 128
            o = 0
            while o < total:
                n = min(2048, total - o)
                zds.append(nc.sync.dma_start(out=flat[:, o:o + n], in_=zt[:, :n]))
                o += n

            # gather/scatter chunks
            off = 0
            last_sc = None
            first = True
            for (h, n) in chunks:
                assert n == CHUNK_MAX
                gt = gp.tile([128, CHUNK_MAX // 128, D], dt, tag="gt")
                cgi = gp.tile([128, CHUNK_MAX // 16], i16, tag="cgi")
                csi = gp.tile([128, CHUNK_MAX // 16], i16, tag="csi")
                c1 = nc.vector.tensor_copy(cgi[:], gix[:, off:off + n // 16])
                c2 = nc.vector.tensor_copy(csi[:], six[:, off:off + n // 16])
                gate(last_sc, c1)
                if first:
                    gate(dg1, dg2, *zds)
                    first = False
                g = nc.gpsimd.dma_gather(
                    gt[:],
                    srca[XA_LO:XA_ROWS, :] if h else srca[0:XA_LO, :],
                    cgi[:], n, n, D)
                gate(g, c2)
                last_sc = nc.gpsimd.dma_scatter_add(
                    acc[:], gt[:], csi[:], n, n, D)
                off += n // 16

            # fold + normalize -> hop tiles; write hop_sl
            hop_tiles = []
            gate(last_sc)
            accv = acc[:].rearrange("(s r) d -> s r d", s=NSLOT)
            for t in range(NT):
                ft = hp.tile([128, NSLOT, D], dt, tag="fold")
                nc.sync.dma_start(
                    out=ft[:],
                    in_=accv[:, t * 128:(t + 1) * 128, :].rearrange("s r d -> r s d"))
                ht = pc.tile([128, D], dt, tag=f"h_{t}")
                nc.vector.tensor_tensor(out=ht[:], in0=ft[:, 0, :], in1=ft[:, 1, :],
                                        op=mybir.AluOpType.add)
                nc.vector.tensor_tensor(out=ht[:], in0=ht[:], in1=ft[:, 2, :],
                                        op=mybir.AluOpType.add)
                nc.vector.tensor_tensor(out=ht[:], in0=ht[:], in1=ft[:, 3, :],
                                        op=mybir.AluOpType.add)
                nc.vector.tensor_scalar_mul(ht[:], ht[:], inv_t[:, t:t + 1])
                nc.sync.dma_start(out=hop_h[t * 128:(t + 1) * 128, :], in_=ht[:])
                hop_tiles.append(ht)

            # linear: out = [x | h1_sl | hop] @ W.T + b
            ident = pc.tile([128, 128], dt)
            nc.sync.dma_start(out=ident[:], in_=ident_h[:])
            wt_t = pc.tile([128, 3, D], dt)
            nc.sync.dma_start(out=wt_t[:], in_=wt_h[:].rearrange("(k p) d -> p k d", p=128))
            bias_t = pc.tile([128, D], dt)
            nc.sync.dma_start(out=bias_t[:], in_=bias_h[:])

            for t in range(NT):
                xt = hp.tile([128, D], dt, tag="xt")
                nc.sync.dma_start(out=xt[:], in_=x_sl[t * 128:(t + 1) * 128, :])
                h1t = hp.tile([128, D], dt, tag="h1t")
                nc.sync.dma_start(out=h1t[:], in_=h1_sl[t * 128:(t + 1) * 128, :])
                po = pp.tile([128, D], dt, tag="po")
                for j, ft in enumerate([xt, h1t, hop_tiles[t]]):
                    pt = pp.tile([128, D], dt, tag="pt")
                    nc.tensor.transpose(pt[:], ft[:], ident[:])
                    st = hp.tile([128, D], dt, tag="st")
                    nc.vector.tensor_copy(st[:], pt[:])
                    nc.tensor.matmul(po[:], st[:], wt_t[:, j, :],
                                     start=(j == 0), stop=(j == 2))
                ot = hp.tile([128, D], dt, tag="ot")
                nc.vector.tensor_tensor(out=ot[:], in0=po[:], in1=bias_t[:],
                                        op=mybir.AluOpType.add)
                nc.sync.dma_start(out=out_h[t * 128:(t + 1) * 128, :], in_=ot[:])

    nc.finalize()
    return nc


def kernel(x, edge_index, W, b):
    x = np.asarray(x, np.float32)
    W = np.asarray(W, np.float32)
    b = np.asarray(b, np.float32)
    ekey = hash(np.asarray(edge_index).tobytes())
    if ekey not in _CACHE:
        pre = _prep(edge_index)
        nc = _build(pre["chunks"], len(pre["gidx"][0]))
        _CACHE.clear()
        _CACHE[ekey] = (pre, nc)
    pre, nc = _CACHE[ekey]

    ident = np.eye(128, dtype=np.float32)
    bias_rep = np.tile(b[None, :], (128, 1)).astype(np.float32)
    wt = np.ascontiguousarray(W.T).astype(np.float32)
    zsl = np.zeros((6272, D), np.float32)

    def run(srca, h1_slices):
        in_maps = []
        for c in range(P):
            x_sl = np.zeros((6272, D), np.float32)
            x_sl[:SH] = x[c * SH:(c + 1) * SH]
            in_maps.append({
                "srca": srca, "x_sl": x_sl,
                "h1_sl": h1_slices[c] if h1_slices is not None else zsl,
                "g_h": _wrap_idx(pre["gidx"][c]), "s_h": _wrap_idx(pre["sidx"][c]),
                "inv_h": pre["invc"][c],
                "wt_h": wt, "bias_h": bias_rep, "ident_h": ident,
            })
        return run_bass_kernel_spmd(nc, in_maps, list(range(P)))

    r1 = run(_aug(x), None)
    h1_slices = [r1.results[c]["hop_sl"] for c in range(P)]
    hop1_full = np.concatenate([s[:SH] for s in h1_slices], axis=0)
    r2 = run(_aug(hop1_full), h1_slices)
    out = np.concatenate([r2.results[c]["out_sl"][:SH] for c in range(P)], axis=0)
    return out.astype(np.float32)
